# revision 28
# baseline (speedup 1.0000x reference)
"""GAT (graph attention network) Bass kernel for 8 trn2 NeuronCores.

Row-sharded: core k owns query rows [k*512, (k+1)*512).  Per j-slab
"variant" schedule balances the N^2 softmax elementwise work across
ScalarE / VectorE / GpSimd / PE:

 - "A" slabs (additive mask {0,-100} in nm): z = nm + s1_i + s2_j via
   batched TT + per-block TS, then ScalarE Prelu+Exp, one matmul vs
   [Wh|1] per block (baseline scheme).
 - "V"/"Vg" slabs (multiplicative mask {1,0} in nm): uses the identity
   exp(leaky(z)) = max(exp z, exp 0.2z) and exp(s1+s2) = E1_i*E2_j.
   The only per-element work is c = [s1_i + s2_j >= 0] (4x-mode TS),
   q = c*M, q2 = M - q (2x-mode TT; q2 on GpSimd for "Vg").  PE then
   computes psU += (E2-scaled WhT) @ q and psV += (F2-scaled WhT) @ q2;
   out = E1_i*psU + F1_i*psV.  No ScalarE work at all in the N^2 path.

Wh for all 8 heads is computed by 512-wide-rhs matmuls per node block
(xTn node-major stationary), cast into per-variant lhsT tiles (plain /
E2-scaled / F2-scaled with the denominator ones/e2/f2 in column 64).
AllGather of Wh_o runs in 3 bf16 chunks pipelined under heads 5-8.
"""

import sys

sys.path.insert(0, "/opt/trn_rl_repo")

import numpy as np
import ml_dtypes

import concourse.bass as bass
import concourse.bacc as bacc
import concourse.tile as tile
from concourse import mybir
from concourse.bass_utils import run_bass_kernel_spmd
from concourse.masks import make_identity

F32 = mybir.dt.float32
BF16 = mybir.dt.bfloat16
BF = ml_dtypes.bfloat16
ADD = mybir.AluOpType.add
SUB = mybir.AluOpType.subtract
MULT = mybir.AluOpType.mult
MAX = mybir.AluOpType.max
MIN = mybir.AluOpType.min
GE = mybir.AluOpType.is_ge
AF = mybir.ActivationFunctionType

# problem shape (hardcoded per spec)
N = 4096
F_IN = 512
O = 64
H = 8
C = 16
N_CORES = 8
NEG = -100.0
ALPHA = 0.2

KF = F_IN // 128   # f_in k-tiles
GROUP = 4          # j-blocks per slab
NB = N // 128      # 32 key blocks
NG = NB // GROUP   # 8 slabs

# per-slab variant schedule (A=prelu/exp path, V=factorized-exp path,
# Vg = V with q2 on GpSimd).  Applied to all 8 hidden heads + output.
VARIANTS = ["A", "A", "A", "A", "V", "V", "Vg", "Vg"]
ACAST_ACT = True   # A-block Wh cast on ScalarE (else VectorE)
E2REP_DMA = True   # e2/f2 repeat tiles via DRAM-bounce broadcast DMA

A_SLABS = [g for g, v in enumerate(VARIANTS) if v.startswith("A")]
V_SLABS = [g for g, v in enumerate(VARIANTS) if v.startswith("V")]
A_BLOCKS = [jb for g in A_SLABS for jb in range(g * GROUP, (g + 1) * GROUP)]
V_BLOCKS = [jb for g in V_SLABS for jb in range(g * GROUP, (g + 1) * GROUP)]
POS_A = {jb: i for i, jb in enumerate(A_BLOCKS)}
POS_V = {jb: i for i, jb in enumerate(V_BLOCKS)}
NA, NV = len(A_BLOCKS), len(V_BLOCKS)


def _build_nc(n_cores=N_CORES, n=N):
    OWN = n // n_cores
    OB = OWN // 128
    nc = bacc.Bacc("TRN2", target_bir_lowering=False, debug=False,
                   num_devices=n_cores)

    d_xTn = nc.dram_tensor("xTn", [128, NB * 512], BF16, kind="ExternalInput")
    d_xo2 = nc.dram_tensor("xo2", [128, 4 * 512], BF16, kind="ExternalInput")
    d_w64 = nc.dram_tensor("w64b", [128, KF * 512], BF16, kind="ExternalInput")
    d_waA = nc.dram_tensor("waA", [128, KF * 16], BF16, kind="ExternalInput")
    d_wo1 = nc.dram_tensor("wo1r", [128, KF * 128], BF16, kind="ExternalInput")
    d_nm = nc.dram_tensor("nmT", [128, NB * OWN], BF16, kind="ExternalInput")
    d_wot = nc.dram_tensor("wot", [128, KF * C], BF16, kind="ExternalInput")
    d_a2o = nc.dram_tensor("a2o", [128, C], F32, kind="ExternalInput")
    d_out = nc.dram_tensor("out", [OWN, C], F32, kind="ExternalOutput")

    with tile.TileContext(nc) as tc:
        with (
            tc.tile_pool(name="dram", bufs=1, space="DRAM") as dram,
            tc.tile_pool(name="const", bufs=1) as const,
            tc.tile_pool(name="work", bufs=2) as work,
            tc.tile_pool(name="small", bufs=2) as small,
            tc.tile_pool(name="bc", bufs=2) as bc,
            tc.tile_pool(name="erep", bufs=4) as erep,
            tc.tile_pool(name="psW", bufs=2, space="PSUM") as psW,
            tc.tile_pool(name="psM", bufs=2, space="PSUM") as psM,
            tc.tile_pool(name="psPh", bufs=1, space="PSUM") as psPh,
        ):
            # ---- input loads (ordered so compute can start early) ----
            waA = const.tile([128, KF * 16], BF16)
            nc.sync.dma_start(out=waA, in_=d_waA[:])
            w64 = const.tile([128, KF * 512], BF16)
            nc.sync.dma_start(out=w64, in_=d_w64[:])
            xo2 = bc.tile([128, 4 * 512], BF16, tag="s1b", name="xo2")
            nc.sync.dma_start(out=xo2, in_=d_xo2[:])
            xTn = const.tile([128, NB * 512], BF16)
            for g in range(NG):
                w_ = GROUP * 512
                nc.sync.dma_start(out=xTn[:, g * w_:(g + 1) * w_],
                                  in_=d_xTn[:, g * w_:(g + 1) * w_])
            nm = const.tile([128, NB * OWN], BF16)
            for g in range(NG):
                w_ = GROUP * OWN
                nc.sync.dma_start(out=nm[:, g * w_:(g + 1) * w_],
                                  in_=d_nm[:, g * w_:(g + 1) * w_])
            wo1 = const.tile([128, KF * 128], BF16)
            nc.sync.dma_start(out=wo1, in_=d_wo1[:])
            wot = const.tile([128, KF * C], BF16)
            nc.sync.dma_start(out=wot, in_=d_wot[:])
            a2o = const.tile([128, C], F32)
            nc.sync.dma_start(out=a2o, in_=d_a2o[:])

            ident = const.tile([128, 128], F32)
            make_identity(nc, ident[:])
            identb = const.tile([16, 16], BF16)
            make_identity(nc, identb[:])

            # ---- persistent lhsT tiles ----
            whtA = whtE = whtF = None
            if NA:
                whtA = const.tile([128, NA * 520], BF16)
                nc.vector.memset(
                    whtA[:].rearrange("p (b h w) -> p b h w", h=H, w=65)
                    [:, :, :, 64:65], 1.0)
            if NV:
                whtE = const.tile([128, NV * 520], BF16)
                whtF = const.tile([128, NV * 520], BF16)
            hT_all = const.tile([128, KF * OWN], BF16)
            s2cols = const.tile([128, NB * 16], F32)
            e2cols = const.tile([128, NB * 8], BF16)
            f2cols = const.tile([128, NB * 8], BF16)
            d_e2 = dram.tile([128, NB * 8], BF16, name="d_e2")
            d_f2 = dram.tile([128, NB * 8], BF16, name="d_f2")

            # ---- s rows for own blocks -> s1T / E1T / F1T, bounce to DRAM
            pso = psM.tile([16, 512], F32, tag="mm", name="pso")
            for k in range(KF):
                rhs = (xo2[:].rearrange("p (b km) -> p b km", km=512)
                       [:, :, k * 128:(k + 1) * 128])
                nc.tensor.matmul(pso[:].rearrange("q (b m) -> q b m", m=128),
                                 waA[:, k * 16:(k + 1) * 16], rhs,
                                 start=(k == 0), stop=(k == KF - 1))
            so_sb = small.tile([16, 512], BF16, tag="sosb", name="so_sb",
                               bufs=1)
            nc.vector.tensor_copy(so_sb[:], pso[:])
            sE = small.tile([16, OWN], BF16, tag="sef", name="sE", bufs=1)
            nc.scalar.activation(sE[:], so_sb[:], AF.Exp)
            sF = small.tile([16, OWN], BF16, tag="sef2", name="sF", bufs=1)
            nc.scalar.activation(sF[:], so_sb[:], AF.Exp, scale=ALPHA)
            d_s1 = dram.tile([24, OWN], BF16, name="d_s1")
            nc.sync.dma_start(out=d_s1[0:8, :], in_=so_sb[8:16, :])
            nc.sync.dma_start(out=d_s1[8:16, :], in_=sE[8:16, :])
            nc.sync.dma_start(out=d_s1[16:24, :], in_=sF[8:16, :])

            # ---- s sweep: one 4-nb chunk -> s2cols (+exp cols) ----
            def emit_s_chunk(ch):
                nb0 = ch * 4
                pss = psM.tile([16, 512], F32, tag="mm", name=f"pss{ch}")
                for k in range(KF):
                    rhs = (xTn[:, nb0 * 512:(nb0 + 4) * 512]
                           .rearrange("p (b km) -> p b km", km=512)
                           [:, :, k * 128:(k + 1) * 128])
                    nc.tensor.matmul(
                        pss[:].rearrange("q (b m) -> q b m", m=128),
                        waA[:, k * 16:(k + 1) * 16], rhs,
                        start=(k == 0), stop=(k == KF - 1))
                sc = small.tile([16, 512], BF16, tag="sc", name=f"sc{ch}")
                nc.vector.tensor_copy(sc[:], pss[:])
                pst = psM.tile([128, 64], BF16, tag="mm", name=f"pst{ch}")
                for u in range(4):
                    nc.tensor.transpose(pst[:, u * 16:(u + 1) * 16],
                                        sc[:, u * 128:(u + 1) * 128],
                                        identb[:])
                nc.vector.tensor_copy(s2cols[:, nb0 * 16:(nb0 + 4) * 16],
                                      pst[:])
                # exp cols for V blocks (harmless for A blocks)
                sv = (s2cols[:, nb0 * 16:(nb0 + 4) * 16]
                      .rearrange("p (b r) -> p b r", r=16)[:, :, 0:8])
                ev = (e2cols[:, nb0 * 8:(nb0 + 4) * 8]
                      .rearrange("p (b r) -> p b r", r=8))
                fv = (f2cols[:, nb0 * 8:(nb0 + 4) * 8]
                      .rearrange("p (b r) -> p b r", r=8))
                nc.scalar.activation(ev, sv, AF.Exp)
                nc.scalar.activation(fv, sv, AF.Exp, scale=ALPHA)
                if E2REP_DMA and V_BLOCKS:
                    nc.sync.dma_start(out=d_e2[:, nb0 * 8:(nb0 + 4) * 8],
                                      in_=e2cols[:, nb0 * 8:(nb0 + 4) * 8])
                    nc.sync.dma_start(out=d_f2[:, nb0 * 8:(nb0 + 4) * 8],
                                      in_=f2cols[:, nb0 * 8:(nb0 + 4) * 8])

            # ---- Wh sweep: all heads per node block, cast by variant ----
            ones65 = const.tile([128, 65], BF16)
            nc.vector.memset(ones65[:], 1.0)

            def emit_wh_block(jb):
                psw = psW.tile([128, 512], F32, tag="whp", name=f"whp{jb}")
                for k in range(KF):
                    nc.tensor.matmul(
                        psw[:], xTn[:, jb * 512 + k * 128: jb * 512 + (k + 1) * 128],
                        w64[:, k * 512:(k + 1) * 512],
                        start=(k == 0), stop=(k == KF - 1))
                pview = psw[:].rearrange("p (h w) -> p h w", w=O)
                if jb in POS_A:
                    pa = POS_A[jb]
                    dst = (whtA[:, pa * 520:(pa + 1) * 520]
                           .rearrange("p (h w) -> p h w", w=65)[:, :, 0:O])
                    if ACAST_ACT:
                        nc.scalar.activation(dst, pview, AF.Copy)
                    else:
                        nc.vector.tensor_copy(dst, pview)
                else:
                    pv = POS_V[jb]
                    for rep, dd, cols in ((d_e2, whtE, e2cols),
                                          (d_f2, whtF, f2cols)):
                        # r laid out [w=65, h=8]: innermost h continuous
                        r = erep.tile([128, 520], BF16, tag="er",
                                      name=f"er{jb}_{dd is whtF}")
                        if E2REP_DMA:
                            nc.sync.dma_start(
                                out=r[:].rearrange("p (w h) -> p w h", h=8),
                                in_=rep[:, jb * 8:(jb + 1) * 8]
                                .rearrange("p (o h) -> p o h", o=1)
                                .to_broadcast([128, 65, 8]))
                        else:
                            for h in range(H):
                                nc.vector.tensor_scalar(
                                    r[:].rearrange("p (w h) -> p h w", h=8)
                                    [:, h:h + 1, :],
                                    ones65[:].rearrange("p (o w) -> p o w",
                                                        o=1),
                                    cols[:, jb * 8 + h: jb * 8 + h + 1],
                                    None, MULT)
                        rview = r[:].rearrange("p (w h) -> p h w", h=8)
                        dst = (dd[:, pv * 520:(pv + 1) * 520]
                               .rearrange("p (h w) -> p h w", w=65))
                        nc.vector.tensor_tensor(
                            dst[:, :, 0:O], pview, rview[:, :, 0:O], MULT)
                        nc.vector.tensor_copy(dst[:, :, 64:65],
                                              rview[:, :, 64:65])

            # ---- broadcast helpers (per head) ----
            def head_bcast(h):
                s1b4 = bc.tile([128, 4 * OWN], BF16, tag="s1b", name=f"s1b{h}")
                nc.sync.dma_start(
                    out=s1b4[:].rearrange("p (o w) -> p o w", w=OWN),
                    in_=d_s1[h: h + 1, :].rearrange("a (o w) -> a o w", o=1)
                    .to_broadcast([128, 4, OWN]))
                E1b = bc.tile([65, OWN], BF16, tag="e1b", name=f"E1b{h}")
                nc.sync.dma_start(out=E1b[:],
                                  in_=d_s1[8 + h: 9 + h, :].to_broadcast([65, OWN]))
                F1b = bc.tile([65, OWN], BF16, tag="f1b", name=f"F1b{h}")
                nc.sync.dma_start(out=F1b[:],
                                  in_=d_s1[16 + h: 17 + h, :].to_broadcast([65, OWN]))
                return s1b4, E1b, F1b

            # ---- attention pass (hidden head or output layer) ----
            def attention(s1b4, s2ptr, lhsA, lhsB_EF, m_rows, psA, psU, psV,
                          tagp, pre_slab=None, premade=None):
                lw = 520 if m_rows == 65 else 17 * H  # unused for output
                firstA = A_BLOCKS[0] if NA else None
                lastA = A_BLOCKS[-1] if NA else None
                firstV = V_BLOCKS[0] if NV else None
                lastV = V_BLOCKS[-1] if NV else None
                for g in range(NG):
                    if pre_slab is not None:
                        pre_slab(g)
                    var = VARIANTS[g]
                    nmslab = nm[:, g * GROUP * OWN:(g + 1) * GROUP * OWN]
                    if var.startswith("A"):
                        if premade and g in premade:
                            zs = premade[g]
                        else:
                            zs = work.tile([128, GROUP * OWN], BF16, tag="z",
                                           name=f"z{tagp}_{g}", bufs=4)
                            nc.vector.tensor_tensor(zs[:], nmslab, s1b4[:],
                                                    ADD)
                        for q in range(GROUP):
                            jb = g * GROUP + q
                            nc.vector.tensor_scalar(
                                zs[:, q * OWN:(q + 1) * OWN],
                                zs[:, q * OWN:(q + 1) * OWN],
                                s2ptr(jb), None, ADD)
                        us = work.tile([128, GROUP * OWN], BF16, tag="p",
                                       name=f"u{tagp}_{g}")
                        nc.scalar.activation(us[:], zs[:], AF.Prelu,
                                             alpha=ALPHA)
                        os_ = work.tile([128, GROUP * OWN], BF16, tag="o",
                                        name=f"o{tagp}_{g}")
                        nc.scalar.activation(os_[:], us[:], AF.Exp)
                        for q in range(GROUP):
                            jb = g * GROUP + q
                            nc.tensor.matmul(
                                psA[0:m_rows, :], lhsA(jb),
                                os_[:, q * OWN:(q + 1) * OWN],
                                start=(jb == firstA), stop=(jb == lastA))
                    else:
                        cs = work.tile([128, GROUP * OWN], BF16, tag="z",
                                       name=f"c{tagp}_{g}", bufs=4)
                        for q in range(GROUP):
                            jb = g * GROUP + q
                            nc.vector.tensor_scalar(
                                cs[:, q * OWN:(q + 1) * OWN],
                                s1b4[:, 0:OWN], s2ptr(jb), 0.0, ADD, GE)
                        qs = work.tile([128, GROUP * OWN], BF16, tag="p",
                                       name=f"q{tagp}_{g}")
                        nc.vector.tensor_tensor(qs[:], cs[:], nmslab, MULT)
                        q2 = work.tile([128, GROUP * OWN], BF16, tag="o",
                                       name=f"q2{tagp}_{g}")
                        eng = nc.gpsimd if var == "Vg" else nc.vector
                        eng.tensor_tensor(q2[:], nmslab, qs[:], SUB)
                        lhsE, lhsF = lhsB_EF
                        for q in range(GROUP):
                            jb = g * GROUP + q
                            nc.tensor.matmul(
                                psU[0:m_rows, :], lhsE(jb),
                                qs[:, q * OWN:(q + 1) * OWN],
                                start=(jb == firstV), stop=(jb == lastV))
                            nc.tensor.matmul(
                                psV[0:m_rows, :], lhsF(jb),
                                q2[:, q * OWN:(q + 1) * OWN],
                                start=(jb == firstV), stop=(jb == lastV))

            # ---- combine + normalize + elu -> hT_all slot ----
            rstage = const.tile([65, OWN], BF16)

            def finalize(m_rows, psA, psU, psV, E1b, F1b, dst_slot, tag):
                mr = m_rows
                comb = rstage[0:mr, :]
                if NV:
                    u_sb = small.tile([mr, OWN], BF16, tag="usb",
                                      name=f"usb{tag}", bufs=1)
                    nc.scalar.activation(u_sb[:], psU[0:mr, :], AF.Copy)
                    v_sb = small.tile([mr, OWN], BF16, tag="vsb",
                                      name=f"vsb{tag}", bufs=1)
                    nc.scalar.activation(v_sb[:], psV[0:mr, :], AF.Copy)
                    t1 = small.tile([mr, OWN], BF16, tag="t1", name=f"t1{tag}",
                                    bufs=1)
                    nc.vector.tensor_tensor(t1[:], u_sb[:], E1b[0:mr, :], MULT)
                    t2 = small.tile([mr, OWN], BF16, tag="t2", name=f"t2{tag}",
                                    bufs=1)
                    nc.vector.tensor_tensor(t2[:], v_sb[:], F1b[0:mr, :], MULT)
                    if NA:
                        nc.vector.tensor_tensor(t2[:], t2[:], psA[0:mr, :], ADD)
                    nc.vector.tensor_tensor(comb, t1[:], t2[:], ADD)
                else:
                    nc.vector.tensor_copy(comb, psA[0:mr, :])
                # reciprocal of denominators via [128, OB] layout
                rd = dram.tile([1, OWN], BF16, name=f"rd{tag}")
                nc.sync.dma_start(out=rd[:], in_=comb[mr - 1:mr, :])
                r128 = small.tile([128, OB], BF16, tag="r128",
                                  name=f"r128{tag}", bufs=1)
                nc.sync.dma_start(
                    out=r128[:],
                    in_=rd[:].rearrange("a (p b) -> (a p) b", p=128))
                with nc.allow_low_precision(reason="softmax denom recip bf16"):
                    nc.vector.reciprocal(r128[:], r128[:])
                rd2 = dram.tile([1, OWN], BF16, name=f"rd2{tag}")
                nc.sync.dma_start(
                    out=rd2[:].rearrange("a (p b) -> (a p) b", p=128),
                    in_=r128[:])
                rb = small.tile([mr - 1, OWN], BF16, tag="rb", name=f"rb{tag}",
                                bufs=1)
                nc.sync.dma_start(out=rb[:],
                                  in_=rd2[:].to_broadcast([mr - 1, OWN]))
                tn = small.tile([mr - 1, OWN], BF16, tag="tn", name=f"tn{tag}",
                                bufs=1)
                nc.vector.tensor_tensor(tn[:], comb[0:mr - 1, :], rb[:], MULT)
                m0 = small.tile([mr - 1, OWN], BF16, tag="m0", name=f"m0{tag}",
                                bufs=1)
                nc.vector.tensor_scalar(m0[:], tn[:], 0.0, None, MIN)
                g_ = small.tile([mr - 1, OWN], BF16, tag="g", name=f"g{tag}",
                                bufs=1)
                nc.scalar.activation(g_[:], m0[:], AF.Exp)
                nc.vector.scalar_tensor_tensor(dst_slot, g_[:], -1.0, tn[:],
                                               ADD, MAX)

            # ---- gather of Wh_o partial sums (bf16 AllGather) ----
            def emit_gather(tag, cs):
                who = const.tile([128, OB * 17], BF16, name=f"who{tag}")
                for ib in range(OB):
                    pw = psM.tile([128, OWN], F32, tag="mm",
                                  name=f"pw{tag}{ib}")
                    for ci, c in enumerate(cs):
                        nc.tensor.matmul(
                            pw[:, 0:C],
                            hT_all[:, c * OWN + ib * 128: c * OWN + (ib + 1) * 128],
                            wot[:, c * C:(c + 1) * C],
                            start=(ci == 0), stop=(ci == len(cs) - 1))
                    nc.vector.tensor_copy(who[:, ib * 17: ib * 17 + C],
                                          pw[:, 0:C])
                    acc = small.tile([128, 1], F32, tag="acc",
                                     name=f"acc{tag}{ib}")
                    tmp = small.tile([128, C], F32, tag="s2tmp",
                                     name=f"s2o{tag}{ib}")
                    nc.vector.scalar_tensor_tensor(
                        tmp[:], pw[:, 0:C], 1.0, a2o[:], MULT, MULT,
                        accum_out=acc[:])
                    nc.vector.tensor_copy(who[:, ib * 17 + 16: ib * 17 + 17],
                                          acc[:])
                ci_ = dram.tile([128, OB * 17], BF16, name=f"cci{tag}")
                co = dram.tile([n_cores * 128, OB * 17], BF16,
                               addr_space="Shared" if n_cores > 1 else "Local",
                               name=f"cco{tag}")
                nc.gpsimd.dma_start(out=ci_[:], in_=who[:])
                if n_cores > 1:
                    nc.gpsimd.collective_compute(
                        "AllGather", mybir.AluOpType.bypass,
                        replica_groups=[list(range(n_cores))],
                        ins=[ci_.opt()], outs=[co.opt()])
                else:
                    nc.gpsimd.dma_start(out=co[:], in_=ci_[:])
                ga = const.tile([128, NB * 17], BF16, name=f"ga{tag}")
                nc.gpsimd.dma_start(
                    out=ga[:], in_=co[:].rearrange("(g p) f -> p g f", p=128))
                return ga

            # ---- pre-loop: s chunk + wh slab 0 ----
            emit_s_chunk(0)
            for jb in range(GROUP):
                emit_wh_block(jb)

            # ---- hidden heads ----
            prev = [None]
            hb0 = head_bcast(0)
            bcasts = {0: hb0}
            gaA = [None]
            gaB = [None]

            def lhsA_h(h):
                return lambda jb: whtA[:, POS_A[jb] * 520 + h * 65:
                                       POS_A[jb] * 520 + h * 65 + 65]

            def lhsE_h(h, t):
                return lambda jb: t[:, POS_V[jb] * 520 + h * 65:
                                    POS_V[jb] * 520 + h * 65 + 65]

            for h in range(H):
                s1b4, E1b, F1b = bcasts.pop(h)
                psA = psPh.tile([65, OWN], F32, tag="phA", name=f"phA{h}",
                                bufs=2) if NA else None
                psU = psPh.tile([65, OWN], F32, tag="phU", name=f"phU{h}") \
                    if NV else None
                psV = psPh.tile([65, OWN], F32, tag="phV", name=f"phV{h}") \
                    if NV else None

                def pre_slab(g, h=h, psA=psA, psU=psU, psV=psV,
                             E1b=E1b, F1b=F1b):
                    if h == 0:
                        if g + 1 < NG:
                            emit_s_chunk(g + 1)
                            for jb in range((g + 1) * GROUP, (g + 2) * GROUP):
                                emit_wh_block(jb)
                    if g == 0 and h + 1 < H:
                        bcasts[h + 1] = head_bcast(h + 1)
                    if g == 1 and prev[0] is not None:
                        ph_, pu_, pv_, E1p, F1p, hp = prev[0]
                        slot = hT_all[(hp % 2) * 64:(hp % 2) * 64 + 64,
                                      (hp // 2) * OWN:(hp // 2 + 1) * OWN]
                        finalize(65, ph_, pu_, pv_, E1p, F1p, slot, f"h{hp}")

                attention(s1b4, lambda jb: s2cols[:, jb * 16 + h: jb * 16 + h + 1],
                          lhsA_h(h), (lhsE_h(h, whtE), lhsE_h(h, whtF)),
                          65, psA, psU, psV, f"h{h}", pre_slab=pre_slab)
                prev[0] = (psA, psU, psV, E1b, F1b, h)
                if h == 4:
                    gaA[0] = emit_gather("A", [0, 1])
                if h == 6:
                    gaB[0] = emit_gather("B", [2])
            ph_, pu_, pv_, E1p, F1p, hp = prev[0]
            finalize(65, ph_, pu_, pv_, E1p, F1p,
                     hT_all[(hp % 2) * 64:(hp % 2) * 64 + 64,
                            (hp // 2) * OWN:(hp // 2 + 1) * OWN], f"h{hp}")

            # ---- output layer prep ----
            s1ob = const.tile([128, OWN], BF16)
            ps1o = psM.tile([128, OWN], F32, tag="mm", name="ps1o")
            for c in range(KF):
                nc.tensor.matmul(ps1o[:], wo1[:, c * 128:(c + 1) * 128],
                                 hT_all[:, c * OWN:(c + 1) * OWN],
                                 start=(c == 0), stop=(c == KF - 1))
            nc.vector.tensor_copy(s1ob[:], ps1o[:])
            E1o = const.tile([17, OWN], BF16)
            nc.scalar.activation(E1o[:], s1ob[0:17, :], AF.Exp)
            F1o = const.tile([17, OWN], BF16)
            nc.scalar.activation(F1o[:], s1ob[0:17, :], AF.Exp, scale=ALPHA)
            s1ob4 = bc.tile([128, 4 * OWN], BF16, tag="s1b", name="s1ob4")
            for rr in range(4):
                nc.vector.tensor_copy(s1ob4[:, rr * OWN:(rr + 1) * OWN],
                                      s1ob[:])

            gaC = emit_gather("C", [3])
            # fill the gather wait: zpre for A slabs
            zpre = {}
            for g in A_SLABS:
                zp = work.tile([128, GROUP * OWN], BF16, tag="z",
                               name=f"zpre{g}", bufs=4)
                nc.vector.tensor_tensor(
                    zp[:], nm[:, g * GROUP * OWN:(g + 1) * GROUP * OWN],
                    s1ob4[:], ADD)
                zpre[g] = zp

            whoall = const.tile([128, NB * 17], BF16)
            nc.vector.tensor_tensor(whoall[:], gaA[0][:], gaB[0][:], ADD)
            nc.vector.tensor_tensor(whoall[:], whoall[:], gaC[:], ADD)

            # output lhsT tiles by variant
            whoA = whoE = whoF = None
            if NA:
                whoA = const.tile([128, NA * 17], BF16)
                for i, jb in enumerate(A_BLOCKS):
                    nc.vector.tensor_copy(
                        whoA[:, i * 17: i * 17 + C],
                        whoall[:, jb * 17: jb * 17 + C])
                nc.vector.memset(
                    whoA[:].rearrange("p (b w) -> p b w", w=17)[:, :, 16:17],
                    1.0)
            e2o = const.tile([128, NB], F32)
            f2o = const.tile([128, NB], F32)
            s2ocols = const.tile([128, NB], F32)
            s2ov = whoall[:].rearrange("p (b w) -> p b w", w=17)[:, :, 16:17]
            nc.vector.tensor_copy(
                s2ocols[:].rearrange("p (b o) -> p b o", o=1), s2ov)
            nc.scalar.activation(
                e2o[:].rearrange("p (b o) -> p b o", o=1), s2ov, AF.Exp)
            nc.scalar.activation(
                f2o[:].rearrange("p (b o) -> p b o", o=1), s2ov, AF.Exp,
                scale=ALPHA)
            if NV:
                whoE = const.tile([128, NV * 17], BF16)
                whoF = const.tile([128, NV * 17], BF16)
                for i, jb in enumerate(V_BLOCKS):
                    nc.vector.tensor_scalar(
                        whoE[:, i * 17: i * 17 + C],
                        whoall[:, jb * 17: jb * 17 + C],
                        e2o[:, jb: jb + 1], None, MULT)
                    nc.vector.tensor_copy(whoE[:, i * 17 + 16: i * 17 + 17],
                                          e2o[:, jb: jb + 1])
                    nc.vector.tensor_scalar(
                        whoF[:, i * 17: i * 17 + C],
                        whoall[:, jb * 17: jb * 17 + C],
                        f2o[:, jb: jb + 1], None, MULT)
                    nc.vector.tensor_copy(whoF[:, i * 17 + 16: i * 17 + 17],
                                          f2o[:, jb: jb + 1])

            # ---- output attention ----
            poA = psPh.tile([65, OWN], F32, tag="phA", name="poA", bufs=2) \
                if NA else None
            poU = psPh.tile([65, OWN], F32, tag="phU", name="poU") if NV else None
            poV = psPh.tile([65, OWN], F32, tag="phV", name="poV") if NV else None
            attention(s1ob4,
                      lambda jb: s2ocols[:, jb: jb + 1],
                      (lambda jb: whoA[:, POS_A[jb] * 17: (POS_A[jb] + 1) * 17])
                      if NA else None,
                      ((lambda jb: whoE[:, POS_V[jb] * 17: (POS_V[jb] + 1) * 17]),
                       (lambda jb: whoF[:, POS_V[jb] * 17: (POS_V[jb] + 1) * 17])),
                      17, poA, poU, poV, "o", premade=zpre)

            # ---- finalize output: comb/normalize/elu into osb ----
            osb = const.tile([17, OWN], F32)
            finalize(17, poA, poU, poV, E1o, F1o, osb[0:16, :], "out")

            # ---- log_softmax + store ----
            es = const.tile([128, OB * C], F32)
            for tt in range(OB):
                ptr_ = psM.tile([128, C], F32, tag="mm", name=f"ptr{tt}")
                nc.tensor.transpose(ptr_[:, 0:C],
                                    osb[0:16, tt * 128:(tt + 1) * 128],
                                    ident[0:16, 0:16])
                nc.vector.tensor_copy(es[:, tt * C:(tt + 1) * C], ptr_[:, 0:C])
            sall = const.tile([128, OB], F32)
            final = const.tile([128, OB * C], F32)
            for tt in range(OB):
                ex = small.tile([128, C], F32, tag="ex", name=f"ex{tt}")
                nc.scalar.activation(ex[:], es[:, tt * C:(tt + 1) * C],
                                     AF.Exp, accum_out=sall[:, tt: tt + 1])
            for tt in range(OB):
                lns = small.tile([128, 1], F32, tag="lns", name=f"lns{tt}")
                nc.scalar.activation(lns[:], sall[:, tt: tt + 1], AF.Ln)
                nc.vector.tensor_scalar(final[:, tt * C:(tt + 1) * C],
                                        es[:, tt * C:(tt + 1) * C],
                                        lns[:], None, SUB)
            nc.sync.dma_start(
                out=d_out[:].rearrange("(b p) c -> p b c", p=128),
                in_=final[:])

    nc.compile()
    return nc


def _pack_inputs(x, adj, W, a, W_out, a_out, n_cores=N_CORES):
    n, f_in = x.shape
    OWN = n // n_cores
    xf = np.asarray(x, np.float32)
    adj = np.asarray(adj)
    Wf = np.asarray(W, np.float32)
    af = np.asarray(a, np.float32)
    Wof = np.asarray(W_out, np.float32)
    aof = np.asarray(a_out, np.float32)

    # xTn[p, nb*512 + k*128 + m] = x[nb*128+m, 128k+p]
    xTn = (xf.reshape(NB, 128, KF, 128).transpose(3, 0, 2, 1)
           .reshape(128, NB * 512).astype(BF))
    # w64b[p, k*512 + h*64 + o] = W[h, 128k+p, o]
    w64b = (Wf.reshape(H, KF, 128, O).transpose(2, 1, 0, 3)
            .reshape(128, KF * 512).astype(BF))
    wa1 = np.einsum("hfo,ho->hf", Wf, af[:, :O])
    wa2 = np.einsum("hfo,ho->hf", Wf, af[:, O:])
    waA = np.concatenate([wa2, wa1], axis=0)  # [16, F]
    waA = waA.T.reshape(KF, 128, 16).transpose(1, 0, 2).reshape(128, KF * 16)
    waA = waA.astype(BF)
    wo1 = Wof @ aof[:C]
    wo1r = np.broadcast_to(
        wo1.reshape(KF, 128).T[:, :, None], (128, KF, 128)
    ).reshape(128, KF * 128).astype(BF)
    wot = (Wof.reshape(KF, 128, C).transpose(1, 0, 2)
           .reshape(128, KF * C).astype(BF))
    a2o = np.broadcast_to(aof[C:], (128, C)).astype(np.float32).copy()

    vmask = np.zeros(NB, bool)
    for jb in V_BLOCKS:
        vmask[jb] = True

    in_maps = []
    for core in range(n_cores):
        rows = slice(core * OWN, (core + 1) * OWN)
        xo2 = xTn[:, core * 4 * 512:(core + 1) * 4 * 512].copy()
        adjT = adj[rows].T > 0  # [N, OWN]
        nmT = np.where(adjT, np.float32(0), np.float32(NEG))
        nmT = nmT.reshape(NB, 128, OWN)
        mult = np.where(adjT, np.float32(1), np.float32(0)).reshape(NB, 128, OWN)
        nmT[vmask] = mult[vmask]
        nmT = nmT.transpose(1, 0, 2).reshape(128, NB * OWN).astype(BF)
        in_maps.append({
            "xTn": xTn, "xo2": xo2, "w64b": w64b, "waA": waA, "wo1r": wo1r,
            "nmT": nmT, "wot": wot, "a2o": a2o,
        })
    return in_maps


_NC_CACHE = {}


def _get_nc(n_cores=N_CORES, n=N):
    key = (n_cores, n)
    if key not in _NC_CACHE:
        _NC_CACHE[key] = _build_nc(n_cores, n)
    return _NC_CACHE[key]


def kernel(x, adj, W, a, W_out, a_out):
    nc = _get_nc()
    in_maps = _pack_inputs(x, adj, W, a, W_out, a_out)
    res = run_bass_kernel_spmd(nc, in_maps, list(range(N_CORES)))
    out = np.concatenate([res.results[c]["out"] for c in range(N_CORES)], axis=0)
    return out.astype(np.float32)
